# revision 1
# baseline (speedup 1.0000x reference)
import numpy as np

# nn_MultiHeadedAttention: B=4, S=2048, D_MODEL=1024, H=16, D_K=64, fp32.
# Sharding: 8 cores = 4 batches x 2 head-groups (8 heads each).
# Per-core: head-group projections (fp32r matmuls), scores computed
# transposed ST[k,q], exp on ACT straight out of 2-bank PSUM reads,
# PV with an appended ones-column so the softmax denominator falls out
# of row 64 of the PV accumulation, PE-broadcast reciprocal, out-proj
# partial y; host sums the two head-group partials and adds bo.

B, S, D, H, DK = 4, 2048, 1024, 16, 64
NCORES = 8
DG = 512  # dims per head-group (8 heads x 64)

_NC_CACHE = {}
LAST_EXEC_NS = None


def _build_nc():
    import concourse.bacc as bacc
    import concourse.tile as tile
    from concourse import mybir

    F32R = mybir.dt.float32r
    F32 = mybir.dt.float32
    EXP = mybir.ActivationFunctionType.Exp

    nc = bacc.Bacc(None, target_bir_lowering=False, debug=True)

    xqT = nc.dram_tensor("xqT", [D, S], F32R, kind="ExternalInput")
    xkT = nc.dram_tensor("xkT", [D, S], F32R, kind="ExternalInput")
    xvT = nc.dram_tensor("xvT", [D, S], F32R, kind="ExternalInput")
    wqT = nc.dram_tensor("wqT", [D, DG], F32R, kind="ExternalInput")
    wkT = nc.dram_tensor("wkT", [D, DG], F32R, kind="ExternalInput")
    wvT = nc.dram_tensor("wvT", [D, DG], F32R, kind="ExternalInput")
    woT = nc.dram_tensor("woT", [DG, D], F32R, kind="ExternalInput")
    bqc = nc.dram_tensor("bqc", [128, 4], F32, kind="ExternalInput")
    bkc = nc.dram_tensor("bkc", [128, 4], F32, kind="ExternalInput")
    bvr = nc.dram_tensor("bvr", [DG], F32, kind="ExternalInput")
    y_d = nc.dram_tensor("y", [S, D], F32R, kind="ExternalOutput")

    import concourse.bass as bass

    with (
        tile.TileContext(nc) as tc,
        nc.allow_low_precision(reason="float32r carries full fp32 bits"),
        tc.tile_pool(name="persist", bufs=1) as persist,
    ):
        QT = [persist.tile([128, S], F32R, name=f"QT{p}") for p in range(4)]
        KT = [persist.tile([128, S], F32R, name=f"KT{p}") for p in range(4)]
        AT = [persist.tile([128, S], F32R, name=f"AT{p}") for p in range(4)]
        vones = [persist.tile([128, 8, 65], F32R, name=f"vones{s}") for s in range(16)]
        bq_sb = persist.tile([128, 4], F32, name="bq_sb")
        bk_sb = persist.tile([128, 4], F32, name="bk_sb")
        bv_sb = persist.tile([128, DG], F32, name="bv_sb")
        ones_row = persist.tile([1, 64], F32R, name="ones_row")

        nc.gpsimd.dma_start(bq_sb[:], bqc[:])
        nc.gpsimd.dma_start(bk_sb[:], bkc[:])
        bv_ap = bvr[:]
        bv_bcast = bass.AP(tensor=bv_ap.tensor, offset=bv_ap.offset, ap=[[0, 128], *bv_ap.ap])
        nc.gpsimd.dma_start(bv_sb[:], bv_bcast)
        nc.vector.memset(ones_row[:].bitcast(F32), 1.0)
        for s in range(16):
            nc.vector.memset(vones[s][:, :, 64:65].bitcast(F32), 1.0)

        # ---- Q, K, V projections (one scope; weight arena rotates so the
        # next phase's weights stream in during the current phase) ----
        with (
            tc.tile_pool(name="proj", bufs=2) as stage,
            tc.tile_pool(name="psproj", bufs=4, space="PSUM") as psp,
        ):
            def load_w(w_d):
                wt = stage.tile([128, 8, DG], F32R, name="wt")
                for i in range(8):
                    nc.gpsimd.dma_start(wt[:, i, :], w_d[i * 128 : (i + 1) * 128, :])
                return wt

            def qk_phase(x_d, b_sb, wt, OUT):
                for qc in range(4):
                    qs = slice(qc * 512, (qc + 1) * 512)
                    xs = stage.tile([128, 8, 512], F32R, name="xs")
                    for i in range(8):
                        eng = nc.sync if i % 2 == 0 else nc.scalar
                        eng.dma_start(xs[:, i, :], x_d[i * 128 : (i + 1) * 128, qs])
                    for p in range(4):
                        pp = psp.tile([128, 512], F32, name="pp")
                        for i in range(8):
                            nc.tensor.matmul(
                                pp[:],
                                wt[:, i, p * 128 : (p + 1) * 128],
                                xs[:, i, :],
                                start=(i == 0),
                                stop=(i == 7),
                            )
                        nc.vector.tensor_scalar_add(OUT[p][:, qs], pp[:], b_sb[:, p : p + 1])

            wq = load_w(wqT)
            wk = load_w(wkT)
            qk_phase(xqT, bq_sb, wq, QT)
            wv = load_w(wvT)  # reuses wq's buffer; WAR on Q reads already recorded
            qk_phase(xkT, bk_sb, wk, KT)
            # V projection (natural layout [s, d] + ones column)
            for sb in range(16):
                ss = slice(sb * 128, (sb + 1) * 128)
                xv = stage.tile([128, 8, 128], F32R, name="xv", bufs=3)
                for i in range(8):
                    eng = nc.sync if i % 2 == 0 else nc.scalar
                    eng.dma_start(xv[:, i, :], xvT[i * 128 : (i + 1) * 128, ss])
                vp = psp.tile([128, DG], F32, name="vp")
                for i in range(8):
                    nc.tensor.matmul(
                        vp[:], xv[:, i, :], wv[:, i, :], start=(i == 0), stop=(i == 7)
                    )
                nc.vector.tensor_add(
                    vones[sb][:, :, 0:64],
                    vp[:].rearrange("p (h d) -> p h d", h=8),
                    bv_sb[:].rearrange("p (h d) -> p h d", h=8),
                )

        # prefetch out-proj weights during attention (gpsimd idle there)
        owp_cm = tc.tile_pool(name="out_w", bufs=1)
        owp = owp_cm.__enter__()
        wo = owp.tile([128, 4, D], F32R, name="wo")
        for p in range(4):
            nc.gpsimd.dma_start(wo[:, p, :], woT[p * 128 : (p + 1) * 128, :])

        # ---- attention: scores -> exp -> PV(+denom) -> normalize ----
        # Per (p, qc): score matmuls packed as head pairs (tile rows 0/64),
        # exp staged to SBUF chunks, PV in uninterrupted runs of 8 per head,
        # normalization software-pipelined one iteration behind.
        with (
            tc.tile_pool(name="attn_sb", bufs=3) as asb,
            tc.tile_pool(name="ps_st", bufs=2, space="PSUM") as ps_st,
            tc.tile_pool(name="ps_pv", bufs=2, space="PSUM") as ps_pv,
        ):
            def emit_norm(state):
                pp, pqs, ppv, prec = state
                bc = [ps_st.tile([128, 2, 512], F32, name="st") for _ in range(2)]
                for h in range(2):
                    nc.tensor.matmul(
                        bc[h][0:64, 0, :], ones_row[:], prec[h][:], start=True, stop=True
                    )
                bcs = [asb.tile([64, 512], F32R, name=f"bcs{h}", bufs=1) for h in range(2)]
                for h in range(2):
                    nc.vector.tensor_copy(bcs[h][:], bc[h][0:64, 0, :])
                for h in range(2):
                    hb = h * 64
                    nc.vector.tensor_mul(
                        AT[pp][hb : hb + 64, pqs], ppv[h][0:64, :], bcs[h][:]
                    )

            prev = None
            for p in range(4):
                for qc in range(4):
                    qs = slice(qc * 512, (qc + 1) * 512)
                    pv = [ps_pv.tile([128, 512], F32, name=f"pv{h}") for h in range(2)]
                    ech_tiles = []

                    def sc_chunk(c):
                        ech = asb.tile([128, 2, 4, 512], F32R, name="ech", bufs=3)
                        ech_tiles.append(ech)
                        for kbi in range(4):
                            kb = c * 4 + kbi
                            st = ps_st.tile([128, 2, 512], F32, name="st")
                            for h in range(2):
                                nc.tensor.matmul(
                                    st[:, h, :],
                                    KT[p][h * 64 : (h + 1) * 64, kb * 128 : (kb + 1) * 128],
                                    QT[p][h * 64 : (h + 1) * 64, qs],
                                    start=True,
                                    stop=True,
                                )
                            nc.scalar.activation(
                                out=ech[:, :, kbi, :], in_=st[:], func=EXP, scale=0.125
                            )

                    def pv_half(half):
                        for h in range(2):
                            hidx = p * 2 + h
                            for kbj in range(8):
                                c, kbi = divmod(kbj, 4)
                                kb = half * 8 + kbj
                                nc.tensor.matmul(
                                    pv[h][0:65, :],
                                    vones[kb][:, hidx, :],
                                    ech_tiles[half * 2 + c][:, h, kbi, :],
                                    start=(half == 0 and kbj == 0),
                                    stop=(half == 1 and kbj == 7),
                                )

                    sc_chunk(0)
                    sc_chunk(1)
                    sc_chunk(2)
                    pv_half(0)
                    if prev is not None:
                        emit_norm(prev)
                        prev = None
                    sc_chunk(3)
                    pv_half(1)
                    rec = []
                    for h in range(2):
                        r = asb.tile([1, 512], F32R, name=f"rec{h}", bufs=2)
                        nc.vector.reciprocal(r[:], pv[h][64:65, :])
                        rec.append(r)
                    prev = (p, qs, pv, rec)
            emit_norm(prev)

        # ---- output projection (partial y for this head-group) ----
        with (
            tc.tile_pool(name="out_y", bufs=3) as oy,
            tc.tile_pool(name="ps_y", bufs=4, space="PSUM") as ps_y,
        ):
            for sb in range(16):
                ss = slice(sb * 128, (sb + 1) * 128)
                ys = oy.tile([128, 2, 512], F32R, name="ys")
                for oc in range(2):
                    yp = ps_y.tile([128, 512], F32, name="yp")
                    for p in range(4):
                        nc.tensor.matmul(
                            yp[:],
                            AT[p][:, ss],
                            wo[:, p, oc * 512 : (oc + 1) * 512],
                            start=(p == 0),
                            stop=(p == 3),
                        )
                    nc.vector.tensor_copy(ys[:, oc, :], yp[:])
                eng = (nc.gpsimd, nc.sync, nc.scalar)[sb % 3]
                eng.dma_start(y_d[ss, :], ys[:])
        owp_cm.__exit__(None, None, None)

    nc.compile()
    return nc


def _get_nc():
    if "nc" not in _NC_CACHE:
        _NC_CACHE["nc"] = _build_nc()
    return _NC_CACHE["nc"]


def kernel(**inputs):
    from concourse import bass_utils

    q, k, v = inputs["query"], inputs["key"], inputs["value"]
    Wq, Wk, Wv, Wo = inputs["Wq"], inputs["Wk"], inputs["Wv"], inputs["Wo"]
    bq, bk, bv, bo = inputs["bq"], inputs["bk"], inputs["bv"], inputs["bo"]

    nc = _get_nc()
    in_maps = []
    for c in range(NCORES):
        b, hg = divmod(c, 2)
        r0 = hg * DG
        rs = slice(r0, r0 + DG)
        in_maps.append(
            {
                "xqT": np.ascontiguousarray(q[b].T),
                "xkT": np.ascontiguousarray(k[b].T),
                "xvT": np.ascontiguousarray(v[b].T),
                "wqT": np.ascontiguousarray(Wq[rs, :].T),
                "wkT": np.ascontiguousarray(Wk[rs, :].T),
                "wvT": np.ascontiguousarray(Wv[rs, :].T),
                "woT": np.ascontiguousarray(Wo[:, rs].T),
                "bqc": np.ascontiguousarray(bq[rs].reshape(4, 128).T),
                "bkc": np.ascontiguousarray(bk[rs].reshape(4, 128).T),
                "bvr": np.ascontiguousarray(bv[rs]),
            }
        )
    import os

    trace = bool(os.environ.get("KERNEL_TRACE"))
    res = bass_utils.run_bass_kernel_spmd(
        nc, in_maps, core_ids=list(range(NCORES)), trace=trace
    )
    global LAST_EXEC_NS
    LAST_EXEC_NS = res.exec_time_ns
    out = np.empty((B, S, D), np.float32)
    for b in range(B):
        out[b] = res.results[2 * b]["y"] + res.results[2 * b + 1]["y"] + bo[None, :]
    return out



# revision 10
# speedup vs baseline: 1.1308x; 1.1308x over previous
import numpy as np

# nn_MultiHeadedAttention: B=4, S=2048, D_MODEL=1024, H=16, D_K=64.
# Sharding: 8 cores = 4 batches x 2 head-groups (8 heads each).
# All matmul operands bf16 (2x the fp32r rate on HW: 216ns vs ~450ns per
# N=512 MM). Scores row-tiled 2-heads-concurrent (contraction 64 on array
# rows 0-63/64-127). Softmax exp on ACT in N=1024 chunks from a
# double-buffered 2-bank PSUM score tile; PV accumulates [65,2,512] with a
# ones-column so row 64 is the denominator; reciprocal_approx_fast + DMA
# partition-broadcast replace the old reciprocal+broadcast-matmul chain.
# Q/K projections for later head-pairs and the output projection are
# emitted as PE filler inside the ACT-bound attention span.

B, S, D, H, DK = 4, 2048, 1024, 16, 64
NCORES = 8
DG = 512  # dims per head-group (8 heads x 64)

_NC_CACHE = {}
LAST_EXEC_NS = None


def _build_nc():
    import concourse.bacc as bacc
    import concourse.tile as tile
    from concourse import mybir
    import concourse.bass as bass

    F32 = mybir.dt.float32
    BF16 = mybir.dt.bfloat16
    EXP = mybir.ActivationFunctionType.Exp

    nc = bacc.Bacc(None, target_bir_lowering=False, debug=True)

    xqT = nc.dram_tensor("xqT", [D, S], BF16, kind="ExternalInput")
    xkT = nc.dram_tensor("xkT", [D, S], BF16, kind="ExternalInput")
    xvT = nc.dram_tensor("xvT", [D, S], BF16, kind="ExternalInput")
    wqT = nc.dram_tensor("wqT", [D, DG], BF16, kind="ExternalInput")
    wkT = nc.dram_tensor("wkT", [D, DG], BF16, kind="ExternalInput")
    wvT = nc.dram_tensor("wvT", [D, DG], BF16, kind="ExternalInput")
    woT = nc.dram_tensor("woT", [DG, D], BF16, kind="ExternalInput")
    bqc = nc.dram_tensor("bqc", [128, 4], F32, kind="ExternalInput")
    bkc = nc.dram_tensor("bkc", [128, 4], F32, kind="ExternalInput")
    bvr = nc.dram_tensor("bvr", [DG], F32, kind="ExternalInput")
    y_d = nc.dram_tensor("y", [S, D], F32, kind="ExternalOutput")
    # DRAM bounce buffer for the reciprocal partition-broadcast (SBUF APs
    # cannot have partition stride 0; DRAM sources can). Two slots, used
    # alternately; same-queue FIFO ordering makes reuse safe regardless.
    dscr = nc.dram_tensor("dscr", [2, 2, 512], F32, kind="Internal")

    with (
        tile.TileContext(nc) as tc,
        nc.allow_low_precision(reason="bf16 kernel, rel-err budget 2e-2"),
        tc.tile_pool(name="persist", bufs=1) as persist,
        tc.tile_pool(name="stage", bufs=1) as stage,
        tc.tile_pool(name="echp", bufs=1) as echp,
        tc.tile_pool(name="normp", bufs=1) as normp,
        tc.tile_pool(name="ysp", bufs=1) as ysp,
    ):
        QT = [persist.tile([128, S], BF16, name=f"QT{p}") for p in range(4)]
        KT = [persist.tile([128, S], BF16, name=f"KT{p}") for p in range(4)]
        AT = [persist.tile([128, S], BF16, name=f"AT{p}") for p in range(4)]
        vones = [persist.tile([128, 8, 65], BF16, name=f"vones{s}") for s in range(16)]
        bq_sb = persist.tile([128, 4], F32, name="bq_sb")
        bk_sb = persist.tile([128, 4], F32, name="bk_sb")
        bv_sb = persist.tile([128, DG], F32, name="bv_sb")
        wq = persist.tile([128, 8, DG], BF16, name="wq")
        wk = persist.tile([128, 8, DG], BF16, name="wk")
        wv = persist.tile([128, 8, DG], BF16, name="wv")
        wo = persist.tile([128, 4, D], BF16, name="wo")

        nc.gpsimd.dma_start(bq_sb[:], bqc[:])
        nc.gpsimd.dma_start(bk_sb[:], bkc[:])
        bv_ap = bvr[:]
        bv_bcast = bass.AP(
            tensor=bv_ap.tensor, offset=bv_ap.offset, ap=[[0, 128], *bv_ap.ap]
        )
        nc.gpsimd.dma_start(bv_sb[:], bv_bcast)
        for i in range(8):
            (nc.sync if i % 2 == 0 else nc.gpsimd).dma_start(
                wk[:, i, :], wkT[i * 128 : (i + 1) * 128, :]
            )
            (nc.gpsimd if i % 2 == 0 else nc.sync).dma_start(
                wq[:, i, :], wqT[i * 128 : (i + 1) * 128, :]
            )
            nc.gpsimd.dma_start(wv[:, i, :], wvT[i * 128 : (i + 1) * 128, :])
        for s in range(16):
            nc.vector.memset(vones[s][:, :, 64:65], 1.0)

        # xk chunks stay resident (bufs=4) so K(p1..3) fillers reuse them.
        xk_cache = {}

        def load_xk(c):
            if c not in xk_cache:
                xs = stage.tile([128, 8, 512], BF16, name="xkr", bufs=4)
                for i in range(8):
                    eng = nc.sync if i % 2 == 0 else nc.gpsimd
                    eng.dma_start(xs[:, i, :], xkT[i * 128 : (i + 1) * 128, c * 512 : (c + 1) * 512])
                xk_cache[c] = xs
            return xk_cache[c]

        def load_xq(c):
            xs = stage.tile([128, 8, 512], BF16, name="xs", bufs=2)
            for i in range(8):
                eng = nc.sync if i % 2 == 0 else nc.gpsimd
                eng.dma_start(xs[:, i, :], xqT[i * 128 : (i + 1) * 128, c * 512 : (c + 1) * 512])
            return xs

        def proj_into(psum_ap, xs, w_sb, p4):
            for i in range(8):
                nc.tensor.matmul(
                    psum_ap,
                    w_sb[:, i, p4 * 128 : (p4 + 1) * 128],
                    xs[:, i, :],
                    start=(i == 0),
                    stop=(i == 7),
                )

        # ---- head phase: K[p0] (4 chunks), Q[p0,qc0], V (all 16 sb) ----
        with tc.tile_pool(name="psh", bufs=4, space="PSUM") as psh:
            for c in range(4):
                xs = load_xk(c)
                pp = psh.tile([128, 512], F32, name="pp")
                proj_into(pp[:], xs, wk, 0)
                nc.vector.tensor_scalar_add(
                    KT[0][:, c * 512 : (c + 1) * 512], pp[:], bk_sb[:, 0:1]
                )
            xsq = load_xq(0)
            pp = psh.tile([128, 512], F32, name="pp")
            proj_into(pp[:], xsq, wq, 0)
            nc.vector.tensor_scalar_add(QT[0][:, 0:512], pp[:], bq_sb[:, 0:1])
            for sb in range(16):
                ss = slice(sb * 128, (sb + 1) * 128)
                xv = stage.tile([128, 8, 128], BF16, name="xv", bufs=3)
                for i in range(8):
                    eng = nc.sync if i % 2 == 0 else nc.gpsimd
                    eng.dma_start(xv[:, i, :], xvT[i * 128 : (i + 1) * 128, ss])
                vp = psh.tile([128, DG], F32, name="vp")
                for i in range(8):
                    nc.tensor.matmul(
                        vp[:], xv[:, i, :], wv[:, i, :], start=(i == 0), stop=(i == 7)
                    )
                nc.vector.tensor_add(
                    vones[sb][:, :, 0:64],
                    vp[:].rearrange("p (h d) -> p h d", h=8),
                    bv_sb[:].rearrange("p (h d) -> p h d", h=8),
                )
            for p4 in range(4):
                nc.sync.dma_start(wo[:, p4, :], woT[p4 * 128 : (p4 + 1) * 128, :])

        # ---- attention phase with filler injection ----
        with (
            tc.tile_pool(name="ps_st", bufs=2, space="PSUM") as stp,
            tc.tile_pool(name="ps_pv", bufs=1, space="PSUM") as pvp,
            tc.tile_pool(name="ps_aux", bufs=1, space="PSUM") as auxp,
        ):
            aux_state = {"tile": None, "slot": 0}

            def get_aux_slice():
                if aux_state["slot"] == 0:
                    aux_state["tile"] = auxp.tile([128, 2, 512], F32, name="aux")
                sl = aux_state["tile"][:, aux_state["slot"], :]
                aux_state["slot"] ^= 1
                return sl

            # filler task list: ("k"/"q", p4, chunk, reuse_prev_stage)
            order = [
                ("q", 0, 1, False),
                ("q", 0, 2, False),
                ("q", 0, 3, False),
                ("k", 1, 0, False), ("k", 1, 1, False), ("k", 1, 2, False), ("k", 1, 3, False),
                ("q", 1, 0, False), ("q", 1, 1, False), ("q", 1, 2, False), ("q", 1, 3, False),
                ("k", 2, 0, False), ("k", 2, 1, False), ("k", 2, 2, False), ("k", 2, 3, False),
                ("q", 2, 0, False), ("q", 3, 0, True),
                ("q", 2, 1, False), ("q", 3, 1, True),
                ("k", 3, 0, False), ("k", 3, 1, False), ("k", 3, 2, False), ("k", 3, 3, False),
                ("q", 2, 2, False), ("q", 3, 2, True),
                ("q", 2, 3, False), ("q", 3, 3, True),
            ]
            flt = {"i": 0, "budget": 0.0, "last_xs": None}

            def pull_filler():
                if flt["i"] >= len(order):
                    return
                kind, p4, c, reuse = order[flt["i"]]
                flt["i"] += 1
                if kind == "k":
                    xs = load_xk(c)
                else:
                    xs = flt["last_xs"] if reuse else load_xq(c)
                    flt["last_xs"] = xs
                sl = get_aux_slice()
                proj_into(sl, xs, wk if kind == "k" else wq, p4)
                out = (KT if kind == "k" else QT)[p4]
                b = bk_sb if kind == "k" else bq_sb
                nc.vector.tensor_scalar_add(
                    out[:, c * 512 : (c + 1) * 512], sl, b[:, p4 : p4 + 1]
                )

            op_state = {}

            def outproj_piece(qc, sbi, k):
                # piece k (0..3): one p4 accumulation step for both oc halves;
                # the aux alloc is held across the 4 pieces (only legal while
                # no other aux consumer interleaves, i.e. during p3 / tail).
                sb = qc * 4 + sbi
                ss = slice(sb * 128, (sb + 1) * 128)
                if k == 0:
                    op_state["aux"] = auxp.tile([128, 2, 512], F32, name="aux")
                aux_t = op_state["aux"]
                for oc in range(2):
                    nc.tensor.matmul(
                        aux_t[:, oc, :],
                        AT[k][:, ss],
                        wo[:, k, oc * 512 : (oc + 1) * 512],
                        start=(k == 0),
                        stop=(k == 3),
                    )
                if k == 3:
                    ys = ysp.tile([128, 2, 512], F32, name="ys", bufs=2)
                    nc.vector.tensor_copy(ys[:], aux_t[:])
                    eng = nc.sync if sb % 2 == 0 else nc.gpsimd
                    eng.dma_start(y_d[ss, :], ys[:])

            def emit_outproj(qc, sbi):
                for k in range(4):
                    outproj_piece(qc, sbi, k)

            norm_ctr = {"i": 0}

            def emit_norm(state):
                p_, qc_, pvt = state
                qs_ = slice(qc_ * 512, (qc_ + 1) * 512)
                avP = normp.tile([64, 2, 512], F32, name="avP", bufs=2)
                nc.vector.tensor_copy(avP[:], pvt[0:64, :, :])
                dnb = normp.tile([1, 2, 512], F32, name="dnb", bufs=2)
                nc.vector.tensor_copy(dnb[:], pvt[64:65, :, :])
                prec = normp.tile([1, 2, 512], F32, name="prec", bufs=2)
                # custom-DVE op misreads PSUM sources; run it SBUF->SBUF
                nc.vector.reciprocal_approx_fast(
                    prec[:].rearrange("p a b -> p (a b)"),
                    dnb[:].rearrange("p a b -> p (a b)"),
                )
                slot = norm_ctr["i"] % 2
                norm_ctr["i"] += 1
                nc.gpsimd.dma_start(dscr[slot : slot + 1, :, :], prec[:])
                bcsf = normp.tile([64, 2, 512], F32, name="bcsf", bufs=2)
                dap = dscr[slot : slot + 1, :, :]
                nc.gpsimd.dma_start(
                    bcsf[:],
                    bass.AP(
                        tensor=dap.tensor, offset=dap.offset, ap=[[0, 64], *dap.ap[1:]]
                    ),
                )
                for h in range(2):
                    nc.vector.tensor_mul(
                        AT[p_][h * 64 : (h + 1) * 64, qs_], avP[:, h, :], bcsf[:, h, :]
                    )

            prev = None
            for p in range(4):
                for qc in range(4):
                    qs = slice(qc * 512, (qc + 1) * 512)
                    if prev is not None:
                        emit_norm(prev)
                    pvt = pvp.tile([65, 2, 512], F32, name="pv")
                    ech_tiles = [None] * 16
                    for c in range(16):  # one kb (128 keys) per chunk
                        stt = stp.tile([128, 2, 512], F32, name="st")
                        for h in range(2):
                            nc.tensor.matmul(
                                stt[:, h, :],
                                KT[p][h * 64 : (h + 1) * 64, c * 128 : (c + 1) * 128],
                                QT[p][h * 64 : (h + 1) * 64, qs],
                                start=True,
                                stop=True,
                            )
                        ech = echp.tile([128, 2, 512], BF16, name="ech", bufs=3)
                        nc.scalar.activation(out=ech[:], in_=stt[:], func=EXP, scale=0.125)
                        ech_tiles[c] = ech
                        if c > 0:
                            cc = c - 1
                            for h in range(2):
                                nc.tensor.matmul(
                                    pvt[0:65, h, :],
                                    vones[cc][:, p * 2 + h, :],
                                    ech_tiles[cc][:, h, :],
                                    start=(cc == 0),
                                    stop=False,
                                )
                        # filler pacing: ~1.4 N=512-slots of spare PE per chunk
                        flt["budget"] += 1.4
                        if (
                            flt["budget"] >= 8.0
                            and flt["i"] < len(order)
                            and not (p == 3 and qc >= 1)
                        ):
                            flt["budget"] -= 8.0
                            pull_filler()
                        if p == 3 and qc >= 1:
                            outproj_piece(qc - 1, c // 8 * 2 + (c % 8) // 4, c % 4)
                    for h in range(2):
                        nc.tensor.matmul(
                            pvt[0:65, h, :],
                            vones[15][:, p * 2 + h, :],
                            ech_tiles[15][:, h, :],
                            start=False,
                            stop=True,
                        )
                    prev = (p, qc, pvt)
            emit_norm(prev)
            while flt["i"] < len(order):
                pull_filler()
            for sbi in range(4):
                emit_outproj(3, sbi)

    nc.compile()
    return nc


def _get_nc():
    if "nc" not in _NC_CACHE:
        _NC_CACHE["nc"] = _build_nc()
    return _NC_CACHE["nc"]


def kernel(**inputs):
    import ml_dtypes
    from concourse import bass_utils

    bf16 = ml_dtypes.bfloat16
    q, k, v = inputs["query"], inputs["key"], inputs["value"]
    Wq, Wk, Wv, Wo = inputs["Wq"], inputs["Wk"], inputs["Wv"], inputs["Wo"]
    bq, bk, bv, bo = inputs["bq"], inputs["bk"], inputs["bv"], inputs["bo"]

    nc = _get_nc()
    in_maps = []
    for c in range(NCORES):
        b, hg = divmod(c, 2)
        r0 = hg * DG
        rs = slice(r0, r0 + DG)
        in_maps.append(
            {
                "xqT": np.ascontiguousarray(q[b].T).astype(bf16),
                "xkT": np.ascontiguousarray(k[b].T).astype(bf16),
                "xvT": np.ascontiguousarray(v[b].T).astype(bf16),
                "wqT": np.ascontiguousarray(Wq[rs, :].T).astype(bf16),
                "wkT": np.ascontiguousarray(Wk[rs, :].T).astype(bf16),
                "wvT": np.ascontiguousarray(Wv[rs, :].T).astype(bf16),
                "woT": np.ascontiguousarray(Wo[:, rs].T).astype(bf16),
                "bqc": np.ascontiguousarray(bq[rs].reshape(4, 128).T),
                "bkc": np.ascontiguousarray(bk[rs].reshape(4, 128).T),
                "bvr": np.ascontiguousarray(bv[rs]),
            }
        )
    import os

    trace = bool(os.environ.get("KERNEL_TRACE"))
    res = bass_utils.run_bass_kernel_spmd(
        nc, in_maps, core_ids=list(range(NCORES)), trace=trace
    )
    global LAST_EXEC_NS
    LAST_EXEC_NS = res.exec_time_ns
    out = np.empty((B, S, D), np.float32)
    for b in range(B):
        out[b] = res.results[2 * b]["y"] + res.results[2 * b + 1]["y"] + bo[None, :]
    return out


# revision 29
# speedup vs baseline: 1.2121x; 1.0720x over previous
import numpy as np

# nn_MultiHeadedAttention: B=4, S=2048, D_MODEL=1024, H=16, D_K=64.
# Sharding: 8 cores = 4 batches x 2 head-groups (8 heads each).
# All matmul operands bf16 (2x the fp32r rate on HW: 216ns vs ~450ns per
# N=512 MM). Scores row-tiled 2-heads-concurrent (contraction 64 on array
# rows 0-63/64-127). Softmax exp on ACT in N=1024 chunks from a
# double-buffered 2-bank PSUM score tile; PV accumulates [65,2,512] with a
# ones-column so row 64 is the denominator; reciprocal_approx_fast + DMA
# partition-broadcast replace the old reciprocal+broadcast-matmul chain.
# Q/K projections for later head-pairs and the output projection are
# emitted as PE filler inside the ACT-bound attention span.

B, S, D, H, DK = 4, 2048, 1024, 16, 64
NCORES = 8
DG = 512  # dims per head-group (8 heads x 64)

_NC_CACHE = {}
LAST_EXEC_NS = None


def _build_nc():
    import concourse.bacc as bacc
    import concourse.tile as tile
    from concourse import mybir
    import concourse.bass as bass

    F32 = mybir.dt.float32
    BF16 = mybir.dt.bfloat16
    EXP = mybir.ActivationFunctionType.Exp

    nc = bacc.Bacc(None, target_bir_lowering=False, debug=True)

    xqT = nc.dram_tensor("xqT", [D, S], BF16, kind="ExternalInput")
    xkT = nc.dram_tensor("xkT", [D, S], BF16, kind="ExternalInput")
    xvT = nc.dram_tensor("xvT", [D, S], BF16, kind="ExternalInput")
    wqT = nc.dram_tensor("wqT", [D, DG], BF16, kind="ExternalInput")
    wkT = nc.dram_tensor("wkT", [D, DG], BF16, kind="ExternalInput")
    wvT = nc.dram_tensor("wvT", [D, DG], BF16, kind="ExternalInput")
    woT = nc.dram_tensor("woT", [DG, D], BF16, kind="ExternalInput")
    bqc = nc.dram_tensor("bqc", [128, 4], F32, kind="ExternalInput")
    bkc = nc.dram_tensor("bkc", [128, 4], F32, kind="ExternalInput")
    bvr = nc.dram_tensor("bvr", [DG], F32, kind="ExternalInput")
    y_d = nc.dram_tensor("y", [S, D], F32, kind="ExternalOutput")
    # DRAM bounce buffers for the reciprocal partition-broadcast (SBUF APs
    # cannot have partition stride 0; DRAM sources can). Each queue gets its
    # own write+read slot pair so both chains stay FIFO-ordered within one
    # queue (cross-queue ordering is not guaranteed).
    BF16_ = mybir.dt.bfloat16
    dscr = nc.dram_tensor("dscr", [4, 1024], BF16_, kind="Internal")

    with (
        tile.TileContext(nc) as tc,
        nc.allow_low_precision(reason="bf16 kernel, rel-err budget 2e-2"),
        tc.tile_pool(name="persist", bufs=1) as persist,
        tc.tile_pool(name="stage", bufs=1) as stage,
        tc.tile_pool(name="echp", bufs=1) as echp,
        tc.tile_pool(name="normp", bufs=1) as normp,
        tc.tile_pool(name="ysp", bufs=1) as ysp,
    ):
        QT = [persist.tile([128, S], BF16, name=f"QT{p}") for p in range(4)]
        KT = [persist.tile([128, S], BF16, name=f"KT{p}") for p in range(4)]
        AT = [persist.tile([128, S], BF16, name=f"AT{p}") for p in range(4)]
        vones = [persist.tile([128, 8, 65], BF16, name=f"vones{s}") for s in range(16)]
        bq_sb = persist.tile([128, 4], F32, name="bq_sb")
        bk_sb = persist.tile([128, 4], F32, name="bk_sb")
        bv_sb = persist.tile([128, DG], F32, name="bv_sb")
        wq = persist.tile([128, 8, DG], BF16, name="wq")
        wk = persist.tile([128, 8, DG], BF16, name="wk")
        wv = persist.tile([128, 8, DG], BF16, name="wv")
        wo = persist.tile([128, 4, D], BF16, name="wo")

        # wk first: K[p0] is the critical path to the first scores/exp.
        for i in range(8):
            (nc.sync if i % 2 == 0 else nc.gpsimd).dma_start(
                wk[:, i, :], wkT[i * 128 : (i + 1) * 128, :]
            )
        nc.gpsimd.dma_start(bq_sb[:], bqc[:])
        nc.gpsimd.dma_start(bk_sb[:], bkc[:])
        bv_ap = bvr[:]
        bv_bcast = bass.AP(
            tensor=bv_ap.tensor, offset=bv_ap.offset, ap=[[0, 128], *bv_ap.ap]
        )
        nc.gpsimd.dma_start(bv_sb[:], bv_bcast)
        ones_row = persist.tile([1, 64], BF16, name="ones_row")
        nc.vector.memset(ones_row[:], 1.0)
        for s in range(16):
            nc.vector.memset(vones[s][:, :, 64:65], 1.0)

        # Batched x loads: one dma_start per pair of 128-row blocks
        # ([128, 2, W] with a 3-level AP) to cut issue-queue time 4x.
        def load_x_pairs(x_d, tile_ap, c, w, engines):
            for i in range(4):
                src = x_d[2 * i * 128 : (2 * i + 2) * 128, c * w : (c + 1) * w]
                sap = src
                ap3 = [[sap.ap[0][0], 128], [sap.ap[0][0] * 128, 2], sap.ap[1]]
                engines[i % len(engines)].dma_start(
                    tile_ap[:, 2 * i : 2 * i + 2, :],
                    bass.AP(tensor=sap.tensor, offset=sap.offset, ap=ap3),
                )

        # xk chunks stay resident (bufs=4) so K(p1..3) fillers reuse them.
        xk_cache = {}

        def load_xk(c, engines=(nc.sync, nc.gpsimd)):
            if c not in xk_cache:
                xs = stage.tile([128, 8, 512], BF16, name="xkr", bufs=4)
                load_x_pairs(xkT, xs, c, 512, engines)
                xk_cache[c] = xs
            return xk_cache[c]

        def load_xq(c, engines=(nc.sync, nc.gpsimd)):
            xs = stage.tile([128, 8, 512], BF16, name="xs", bufs=2)
            load_x_pairs(xqT, xs, c, 512, engines)
            return xs

        def proj_into(psum_ap, xs, w_sb, p4):
            for i in range(8):
                nc.tensor.matmul(
                    psum_ap,
                    w_sb[:, i, p4 * 128 : (p4 + 1) * 128],
                    xs[:, i, :],
                    start=(i == 0),
                    stop=(i == 7),
                )

        # ---- head phase: K[p0]+K[p1], Q[p0,0]+Q[p1,0], V (all 16 sb) ----
        # wq/wv/xv DMAs ride the scalar queue (ACT is idle until the first
        # scores land); wo/biases on vector.
        with tc.tile_pool(name="psh", bufs=4, space="PSUM") as psh:
            for c in range(4):
                xs = load_xk(c)
                if c == 0:
                    for i in range(8):
                        nc.scalar.dma_start(wq[:, i, :], wqT[i * 128 : (i + 1) * 128, :])
                for p4 in (0, 1):
                    pp = psh.tile([128, 512], F32, name="pp")
                    proj_into(pp[:], xs, wk, p4)
                    nc.vector.tensor_scalar_add(
                        KT[p4][:, c * 512 : (c + 1) * 512], pp[:], bk_sb[:, p4 : p4 + 1]
                    )
            for i in range(8):
                nc.scalar.dma_start(wv[:, i, :], wvT[i * 128 : (i + 1) * 128, :])
            xsq = load_xq(0)
            for p4 in (0, 1):
                pp = psh.tile([128, 512], F32, name="pp")
                proj_into(pp[:], xsq, wq, p4)
                nc.vector.tensor_scalar_add(
                    QT[p4][:, 0:512], pp[:], bq_sb[:, p4 : p4 + 1]
                )
            for sb in range(16):
                ss = slice(sb * 128, (sb + 1) * 128)
                xv = stage.tile([128, 8, 128], BF16, name="xv", bufs=3)
                for half in range(2):
                    src = xvT[half * 512 : (half + 1) * 512, ss]
                    ap3 = [[src.ap[0][0], 128], [src.ap[0][0] * 128, 4], src.ap[1]]
                    nc.scalar.dma_start(
                        xv[:, half * 4 : (half + 1) * 4, :],
                        bass.AP(tensor=src.tensor, offset=src.offset, ap=ap3),
                    )
                vp = psh.tile([128, DG], F32, name="vp")
                for i in range(8):
                    nc.tensor.matmul(
                        vp[:], xv[:, i, :], wv[:, i, :], start=(i == 0), stop=(i == 7)
                    )
                nc.vector.tensor_add(
                    vones[sb][:, :, 0:64],
                    vp[:].rearrange("p (h d) -> p h d", h=8),
                    bv_sb[:].rearrange("p (h d) -> p h d", h=8),
                )
            for p4 in range(4):
                nc.sync.dma_start(wo[:, p4, :], woT[p4 * 128 : (p4 + 1) * 128, :])

        # ---- attention phase with filler injection ----
        with (
            tc.tile_pool(name="ps_st", bufs=2, space="PSUM") as stp,
            tc.tile_pool(name="ps_pv", bufs=1, space="PSUM") as pvp,
            tc.tile_pool(name="ps_aux", bufs=1, space="PSUM") as auxp,
        ):
            aux_state = {"tile": None, "slot": 0}

            def get_aux_slice():
                if aux_state["slot"] == 0:
                    aux_state["tile"] = auxp.tile([128, 2, 512], F32, name="aux")
                sl = aux_state["tile"][:, aux_state["slot"], :]
                aux_state["slot"] ^= 1
                return sl

            # filler task list: ("k"/"q", p4, chunk, deadline_iter).
            # deadline = one iter before first use; force-pulled at iter
            # starts, budget-pulled mid-chunk for spreading.
            # (K[p1], Q[p1,0] were folded into the head phase)
            order = [
                ("q", 0, 1, 0),
                ("q", 0, 2, 1),
                ("q", 0, 3, 2),
                ("q", 1, 1, 4),
                ("q", 1, 2, 5),
                ("q", 1, 3, 6),
                ("k", 2, 0, 6), ("k", 2, 1, 6), ("k", 2, 2, 7), ("k", 2, 3, 7),
                ("q", 2, 0, 7),
                ("q", 2, 1, 8),
                ("q", 2, 2, 9),
                ("q", 2, 3, 10),
                ("k", 3, 0, 10), ("k", 3, 1, 10), ("k", 3, 2, 11), ("k", 3, 3, 11),
                ("q", 3, 0, 11),
                ("q", 3, 1, 12),
                ("q", 3, 2, 13),
                ("q", 3, 3, 14),
            ]
            flt = {"i": 0, "budget": 0.0}

            def pull_filler():
                if flt["i"] >= len(order):
                    return
                kind, p4, c, _dl = order[flt["i"]]
                flt["i"] += 1
                if kind == "k":
                    xs = load_xk(c)
                else:
                    xs = load_xq(c)
                sl = get_aux_slice()
                proj_into(sl, xs, wk if kind == "k" else wq, p4)
                out = (KT if kind == "k" else QT)[p4]
                b = bk_sb if kind == "k" else bq_sb
                nc.vector.tensor_scalar_add(
                    out[:, c * 512 : (c + 1) * 512], sl, b[:, p4 : p4 + 1]
                )

            op_state = {}

            def outproj_piece(qc, sbi, k):
                # piece k (0..3): one p4 accumulation step for both oc halves;
                # the aux alloc is held across the 4 pieces (only legal while
                # no other aux consumer interleaves, i.e. during p3 / tail).
                sb = qc * 4 + sbi
                ss = slice(sb * 128, (sb + 1) * 128)
                if k == 0:
                    op_state["aux"] = auxp.tile([128, 2, 512], F32, name="aux")
                aux_t = op_state["aux"]
                for oc in range(2):
                    nc.tensor.matmul(
                        aux_t[:, oc, :],
                        AT[k][:, ss],
                        wo[:, k, oc * 512 : (oc + 1) * 512],
                        start=(k == 0),
                        stop=(k == 3),
                    )
                if k == 3:
                    ys = ysp.tile([128, 2, 512], F32, name="ys", bufs=2)
                    nc.vector.tensor_copy(ys[:], aux_t[:])
                    eng = nc.sync if sb % 2 == 0 else nc.gpsimd
                    eng.dma_start(y_d[ss, :], ys[:])

            def emit_outproj(qc, sbi):
                for k in range(4):
                    outproj_piece(qc, sbi, k)

            norm_ctr = {"i": 0}

            def emit_norm(state):
                p_, qc_, pvt = state
                qs_ = slice(qc_ * 512, (qc_ + 1) * 512)
                avP = normp.tile([64, 2, 512], F32, name="avP", bufs=2)
                nc.vector.tensor_copy(avP[:], pvt[0:64, :, :])
                dnb = normp.tile([1, 2, 512], F32, name="dnb", bufs=2)
                nc.vector.tensor_copy(dnb[:], pvt[64:65, :, :])
                prec = normp.tile([1, 2, 512], F32, name="prec", bufs=2)
                # custom-DVE op misreads PSUM sources; run it SBUF->SBUF
                nc.vector.reciprocal_approx_fast(
                    prec[:].rearrange("p a b -> p (a b)"),
                    dnb[:].rearrange("p a b -> p (a b)"),
                )
                precb = normp.tile([1, 2, 512], BF16, name="precb", bufs=2)
                nc.vector.tensor_copy(precb[:], prec[:])
                slot = norm_ctr["i"] % 2
                norm_ctr["i"] += 1
                bcsf = normp.tile([64, 2, 512], BF16, name="bcsf", bufs=2)
                for qi, eng in enumerate((nc.sync, nc.gpsimd)):
                    r = slot * 2 + qi
                    eng.dma_start(
                        dscr[r : r + 1, :], precb[:].rearrange("p a b -> p (a b)")
                    )
                    dap = dscr[r : r + 1, :]
                    eng.dma_start(
                        bcsf[qi * 32 : (qi + 1) * 32, :, :].rearrange(
                            "p a b -> p (a b)"
                        ),
                        bass.AP(
                            tensor=dap.tensor,
                            offset=dap.offset,
                            ap=[[0, 32], *dap.ap[1:]],
                        ),
                    )
                for h in range(2):
                    nc.vector.tensor_mul(
                        AT[p_][h * 64 : (h + 1) * 64, qs_], avP[:, h, :], bcsf[:, h, :]
                    )

            prev = None
            for p in range(4):
                for qc in range(4):
                    it_idx = p * 4 + qc
                    # deadline enforcement: everything this iter needs must
                    # already be emitted
                    while flt["i"] < len(order) and order[flt["i"]][3] <= it_idx:
                        pull_filler()
                    qs = slice(qc * 512, (qc + 1) * 512)
                    if prev is not None:
                        emit_norm(prev)
                    pvt = pvp.tile([65, 2, 512], F32, name="pv")
                    ech_tiles = [None] * 16
                    for c in range(16):  # one kb (128 keys) per chunk
                        stt = stp.tile([128, 2, 512], F32, name="st")
                        for h in range(2):
                            nc.tensor.matmul(
                                stt[:, h, :],
                                KT[p][h * 64 : (h + 1) * 64, c * 128 : (c + 1) * 128],
                                QT[p][h * 64 : (h + 1) * 64, qs],
                                start=True,
                                stop=True,
                            )
                        ech = echp.tile([128, 2, 512], BF16, name="ech", bufs=3)
                        nc.scalar.activation(out=ech[:], in_=stt[:], func=EXP, scale=0.125)
                        ech_tiles[c] = ech
                        if c > 0:
                            cc = c - 1
                            for h in range(2):
                                nc.tensor.matmul(
                                    pvt[0:65, h, :],
                                    vones[cc][:, p * 2 + h, :],
                                    ech_tiles[cc][:, h, :],
                                    start=(cc == 0),
                                    stop=False,
                                )
                        # filler pacing: ~1.4 N=512-slots of spare PE per chunk
                        flt["budget"] += 1.0
                        if (
                            flt["budget"] >= 8.0
                            and flt["i"] < len(order)
                            and not (p == 3 and qc >= 1)
                        ):
                            flt["budget"] -= 8.0
                            pull_filler()
                        # out-projection pieces, shifted +4 chunks so the
                        # AT[3] read lands well after this iter's norm muls
                        if p == 3 and qc >= 1:
                            if c < 4:
                                if qc >= 2:
                                    outproj_piece(qc - 2, 3, c)
                            else:
                                outproj_piece(qc - 1, (c - 4) // 4, (c - 4) % 4)
                    for h in range(2):
                        nc.tensor.matmul(
                            pvt[0:65, h, :],
                            vones[15][:, p * 2 + h, :],
                            ech_tiles[15][:, h, :],
                            start=False,
                            stop=True,
                        )
                    prev = (p, qc, pvt)
            while flt["i"] < len(order):
                pull_filler()
            # final (p3,qc3) norm: pv reads must be emitted before the
            # attention pools close; the broadcast runs on PE in the tail
            # pool (lower latency than the DRAM bounce).
            p_, qc_, pvt = prev
            t_qs = slice(qc_ * 512, (qc_ + 1) * 512)
            t_avP = normp.tile([64, 2, 512], F32, name="avP", bufs=2)
            nc.vector.tensor_copy(t_avP[:], pvt[0:64, :, :])
            t_dnb = normp.tile([1, 2, 512], F32, name="dnb", bufs=2)
            nc.vector.tensor_copy(t_dnb[:], pvt[64:65, :, :])
            t_prec = normp.tile([1, 2, 512], F32, name="prec", bufs=2)
            nc.vector.reciprocal_approx_fast(
                t_prec[:].rearrange("p a b -> p (a b)"),
                t_dnb[:].rearrange("p a b -> p (a b)"),
            )
            t_precb = normp.tile([1, 2, 512], BF16, name="precb", bufs=2)
            nc.vector.tensor_copy(t_precb[:], t_prec[:])

        # ---- tail: PE-broadcast final norm + out-projection qc2-sb3, qc3 ----
        with tc.tile_pool(name="ps_tail", bufs=2, space="PSUM") as tlp:
            bc = tlp.tile([128, 2, 512], F32, name="yp")
            for h in range(2):
                nc.tensor.matmul(
                    bc[0:64, h, :], ones_row[:], t_precb[:, h, :], start=True, stop=True
                )
            t_bcs = normp.tile([64, 2, 512], BF16, name="bcsf", bufs=2)
            nc.vector.tensor_copy(t_bcs[:], bc[0:64, :, :])
            for h in range(2):
                nc.vector.tensor_mul(
                    AT[3][h * 64 : (h + 1) * 64, t_qs], t_avP[:, h, :], t_bcs[:, h, :]
                )
            for qc_o, sbi in [(2, 3), (3, 0), (3, 1), (3, 2), (3, 3)]:
                sb = qc_o * 4 + sbi
                ss = slice(sb * 128, (sb + 1) * 128)
                yp = tlp.tile([128, 2, 512], F32, name="yp")
                for oc in range(2):
                    for p4 in range(4):
                        nc.tensor.matmul(
                            yp[:, oc, :],
                            AT[p4][:, ss],
                            wo[:, p4, oc * 512 : (oc + 1) * 512],
                            start=(p4 == 0),
                            stop=(p4 == 3),
                        )
                ys = ysp.tile([128, 2, 512], F32, name="ys", bufs=2)
                nc.vector.tensor_copy(ys[:], yp[:])
                nc.sync.dma_start(y_d[ss, :], ys[:])

    nc.compile()
    return nc


def _get_nc():
    if "nc" not in _NC_CACHE:
        _NC_CACHE["nc"] = _build_nc()
    return _NC_CACHE["nc"]


def kernel(**inputs):
    import ml_dtypes
    from concourse import bass_utils

    bf16 = ml_dtypes.bfloat16
    q, k, v = inputs["query"], inputs["key"], inputs["value"]
    Wq, Wk, Wv, Wo = inputs["Wq"], inputs["Wk"], inputs["Wv"], inputs["Wo"]
    bq, bk, bv, bo = inputs["bq"], inputs["bk"], inputs["bv"], inputs["bo"]

    nc = _get_nc()
    in_maps = []
    for c in range(NCORES):
        b, hg = divmod(c, 2)
        r0 = hg * DG
        rs = slice(r0, r0 + DG)
        in_maps.append(
            {
                "xqT": np.ascontiguousarray(q[b].T).astype(bf16),
                "xkT": np.ascontiguousarray(k[b].T).astype(bf16),
                "xvT": np.ascontiguousarray(v[b].T).astype(bf16),
                "wqT": np.ascontiguousarray(Wq[rs, :].T).astype(bf16),
                "wkT": np.ascontiguousarray(Wk[rs, :].T).astype(bf16),
                "wvT": np.ascontiguousarray(Wv[rs, :].T).astype(bf16),
                "woT": np.ascontiguousarray(Wo[:, rs].T).astype(bf16),
                "bqc": np.ascontiguousarray(bq[rs].reshape(4, 128).T),
                "bkc": np.ascontiguousarray(bk[rs].reshape(4, 128).T),
                "bvr": np.ascontiguousarray(bv[rs]),
            }
        )
    import os

    trace = bool(os.environ.get("KERNEL_TRACE"))
    res = bass_utils.run_bass_kernel_spmd(
        nc, in_maps, core_ids=list(range(NCORES)), trace=trace
    )
    global LAST_EXEC_NS
    LAST_EXEC_NS = res.exec_time_ns
    out = np.empty((B, S, D), np.float32)
    for b in range(B):
        out[b] = res.results[2 * b]["y"] + res.results[2 * b + 1]["y"] + bo[None, :]
    return out
